# revision 4
# baseline (speedup 1.0000x reference)
"""Trainium2 Bass kernel for a sparse-attention EncoderLayer.

Sharding: rows (L) split into 8 contiguous shards of L/8; each edge is owned
by the core that owns its destination row (row_index is sorted, so each
core's edges are a contiguous range).  Each core computes Q/K/V for its row
shard; K/V shards are AllGathered (bf16, Shared output, in chunks) so every
core holds the full K/V table in HBM; per-edge K/V rows are fetched with
dma_gather; per-edge Q rows come from a one-hot PE matmul against the
SBUF-resident Q table.  Segment softmax runs without max-subtraction
(scores are bounded, exp cannot overflow in f32).  One-hot row selectors
are precomputed on the host.

v2 layout relative to the first version:
  - LN gamma/beta folded into Wq/Wk/Wv/W1 host-side: the on-chip LN is just
    (x - mu) * rstd.
  - Block tails (att norm, Wo, residual, LN2, MLP) are deferred to a phase C
    after the edge loop: the edge phase evicts the scatter PSUM to DRAM and
    runs a pure Copy/Exp Act stream (no act-table thrash), and phase C runs
    the dense matmuls back-to-back (PE stays at high p-state).
  - CHUNK_T=8 edge tiles per gather chunk; software pipeline with the exp
    stage deferred by one chunk so no engine head-of-line blocks another.
"""

import math
import numpy as np
from contextlib import ExitStack

from ml_dtypes import bfloat16

import concourse.bass as bass
import concourse.mybir as mybir
import concourse.tile as tile
from concourse import bacc
from concourse.bass_utils import run_bass_kernel_spmd
from concourse.masks import make_identity

NCORES = 8
C, H, D, HID = 512, 8, 64, 1024
EPS = 1e-5
CHUNK_T = 8   # edge tiles (of 128 edges) per dma_gather chunk
NAG = 8       # allgather chunks
F32 = mybir.dt.float32
BF16 = mybir.dt.bfloat16
I16 = mybir.dt.int16
AF = mybir.ActivationFunctionType
ALU = mybir.AluOpType
AX = mybir.AxisListType

_prog_cache = {}
TRACE = False
LAST_EXEC_NS = None
LAST_RESULTS = None


# --------------------------------------------------------------------------
# host-side preprocessing
# --------------------------------------------------------------------------

def _nag(NBLK):
    return NAG if NBLK % NAG == 0 else 1


def _wrap_idx(idx):
    """[n] int -> [128, n//16] int16, wrapped (idx i at partition i%16,
    column i//16) and replicated across the 8 Q7 cores."""
    n = idx.shape[0]
    w = np.ascontiguousarray(idx.reshape(n // 16, 16).T).astype(np.int16)
    return np.tile(w, (8, 1))


def _preprocess_edges(L, row, col, att_bias):
    LSH = L // NCORES
    NBLK = LSH // 128
    bounds = np.searchsorted(row, np.arange(NCORES + 1) * LSH)

    per_core = []
    t_blk = 1
    for c in range(NCORES):
        e0, e1 = int(bounds[c]), int(bounds[c + 1])
        r = row[e0:e1] - c * LSH
        blk = r >> 7
        cnt = np.bincount(blk, minlength=NBLK)
        t_blk = max(t_blk, int(np.max((cnt + 127) // 128)) if len(cnt) else 1)
        per_core.append((e0, e1, r, blk, cnt))

    T_BLK = t_blk
    NT = NBLK * T_BLK
    NCH = (NT + CHUNK_T - 1) // CHUNK_T
    NTP = NCH * CHUNK_T
    LSH4 = LSH // _nag(NBLK)

    cores = []
    for c in range(NCORES):
        e0, e1, r, blk, cnt = per_core[c]
        ne = e1 - e0
        starts = np.zeros(NBLK, dtype=np.int64)
        np.cumsum(cnt[:-1], out=starts[1:])

        npad = NTP * 128
        # col: global node id -> kv_full row (allgather chunk-major layout)
        gcol = col[e0:e1]
        oc, loc = gcol // LSH, gcol % LSH
        kvrow = (loc // LSH4) * (NCORES * LSH4) + oc * LSH4 + (loc % LSH4)
        # order edges within each block by kv row: improves gather locality
        # and lets early chunks depend on only a prefix of the allgather
        perm = np.lexsort((kvrow, blk))
        blk_s = blk[perm]
        kvrow_s = kvrow[perm]
        idx_in_blk = np.arange(ne, dtype=np.int64) - starts[blk_s]
        dst = blk_s * (T_BLK * 128) + idx_in_blk

        colP = np.zeros(npad, dtype=np.int64)
        rlocP = np.zeros(npad, dtype=np.int64)
        biasP = np.full((npad, H), -30000.0, dtype=np.float32)
        colP[dst] = kvrow_s
        rlocP[dst] = r[perm] & 127
        biasP[dst] = att_bias[e0:e1][perm]
        # per-chunk upper bound on referenced kv rows (for partial AG deps)
        maxrow = colP.reshape(NCH, CHUNK_T * 128).max(axis=1) + 1

        colw = _wrap_idx(colP).reshape(128, NCH, CHUNK_T * 8).transpose(1, 0, 2)
        colw = colw.reshape(NCH * 128, CHUNK_T * 8)
        # one-hot row selector per edge, chunk-partition-major for contiguous
        # DMA: oh[t, e, r] (scatter lhsT); only real edges are set.
        ohu = np.zeros((NTP * 128, 128), dtype=np.uint16)
        ohu[dst, rlocP[dst]] = 0x3F80  # bf16 1.0
        oh = (ohu.view(bfloat16).reshape(NCH, CHUNK_T, 128, 128)
              .transpose(0, 2, 1, 3).reshape(NCH, 128, CHUNK_T * 128))
        # ohT[t, r, e]: row-partition (q-gather lhsT); set for ALL padded
        # slots too (col 0 row 0) so no garbage — padded p is 0 via bias.
        e_in_t = np.arange(npad, dtype=np.int64) % 128
        ohTu = np.zeros((NTP * 128, 128), dtype=np.uint16)
        ohTu[(np.arange(npad) // 128) * 128 + rlocP, e_in_t] = 0x3F80
        ohT = (ohTu.view(bfloat16).reshape(NCH, CHUNK_T, 128, 128)
               .transpose(0, 2, 1, 3).reshape(NCH, 128, CHUNK_T * 128))
        # bias, chunk-partition-major bf16: [NCH, 128, CHUNK_T*H]
        biasT = (biasP.reshape(NCH, CHUNK_T, 128, H).transpose(0, 2, 1, 3)
                 .reshape(NCH, 128, CHUNK_T * H).astype(bfloat16))
        cores.append(dict(
            colw=np.ascontiguousarray(colw),
            biasP=np.ascontiguousarray(biasT),
            ohP=np.ascontiguousarray(oh),
            ohTP=np.ascontiguousarray(ohT),
        ))
        cores[-1]["_maxrow"] = maxrow
    # chunk AG-dep bound must be identical across cores (same program):
    maxrow_all = np.max([c.pop("_maxrow") for c in cores], axis=0)
    return T_BLK, NT, NCH, [int(x) for x in maxrow_all], cores


def _prep_weights(inp):
    scale = 1.0 / math.sqrt(D)
    g1 = np.asarray(inp["ln1_g"], np.float32)
    b1 = np.asarray(inp["ln1_b"], np.float32)
    g2 = np.asarray(inp["ln2_g"], np.float32)
    b2 = np.asarray(inp["ln2_b"], np.float32)

    def mat(w, kchunks):
        w = np.asarray(w, np.float32)
        k, n = w.shape
        assert k == kchunks * 128
        return np.ascontiguousarray(
            w.reshape(kchunks, 128, n).transpose(1, 0, 2)).astype(bfloat16)

    def rowv(b):
        return np.asarray(b, np.float32)[None, :].astype(bfloat16)

    Wq = np.asarray(inp["Wq"], np.float32)
    Wk = np.asarray(inp["Wk"], np.float32)
    Wv = np.asarray(inp["Wv"], np.float32)
    W1 = np.asarray(inp["W1"], np.float32)

    # LN gamma/beta folded into the projections (z = xn*g + b):
    #   z @ W + bw  ==  xn @ (g[:,None]*W)  +  (b @ W + bw)
    return dict(
        wq=mat(g1[:, None] * Wq * scale, 4),
        wk=mat(g1[:, None] * Wk, 4),
        wv=mat(g1[:, None] * Wv, 4),
        wo=mat(inp["Wo"], 4),
        w1=mat(g2[:, None] * W1, 4),
        w2=mat(inp["W2"], 8),
        bq=rowv((b1 @ Wq + np.asarray(inp["bq"], np.float32)) * scale),
        bk=rowv(b1 @ Wk + np.asarray(inp["bk"], np.float32)),
        bv=rowv(b1 @ Wv + np.asarray(inp["bv"], np.float32)),
        bo=rowv(inp["bo"]),
        b1=rowv(b2 @ W1 + np.asarray(inp["b1"], np.float32)),
        b2=rowv(inp["b2"]),
    )


# --------------------------------------------------------------------------
# walrus workaround: split Drain instructions carrying >1 sem wait
# --------------------------------------------------------------------------

def _split_multi_waits(nc):
    nid = [0]
    for fn in nc.m.functions:
        for blk in fn.blocks:
            insts = blk.instructions
            i = 0
            while i < len(insts):
                inst = insts[i]
                si = inst.sync_info
                if (isinstance(inst, mybir.InstDrain)
                        and si is not None and si.on_wait and len(si.on_wait) > 1):
                    waits = list(si.on_wait)
                    nops = []
                    for w in waits[:-1]:
                        nid[0] += 1
                        nops.append(mybir.InstNoOp(
                            name=f"I-waitfix-{nid[0]}",
                            engine=inst.engine, ins=[], outs=[],
                            sync_info=mybir.SyncInfo(on_wait=[w], on_update=[]),
                        ))
                    inst.sync_info = mybir.SyncInfo(
                        on_wait=[waits[-1]], on_update=list(si.on_update))
                    insts[i:i] = nops
                    i += len(nops)
                i += 1


# --------------------------------------------------------------------------
# device program
# --------------------------------------------------------------------------

def _build_program(L, T_BLK, NT, NCH, maxrow):
    LSH = L // NCORES
    NBLK = LSH // 128
    nag = _nag(NBLK)
    LSH4 = LSH // nag
    BPA = NBLK // nag  # blocks per allgather chunk
    nc = bacc.Bacc(num_devices=NCORES)

    x_c = nc.declare_dram_parameter("x_c", [LSH, C], F32, isOutput=False)
    wq = nc.declare_dram_parameter("wq", [128, 4, C], BF16, isOutput=False)
    wk = nc.declare_dram_parameter("wk", [128, 4, C], BF16, isOutput=False)
    wv = nc.declare_dram_parameter("wv", [128, 4, C], BF16, isOutput=False)
    wo = nc.declare_dram_parameter("wo", [128, 4, C], BF16, isOutput=False)
    w1 = nc.declare_dram_parameter("w1", [128, 4, HID], BF16, isOutput=False)
    w2 = nc.declare_dram_parameter("w2", [128, 8, C], BF16, isOutput=False)
    bqp = nc.declare_dram_parameter("bq", [1, C], BF16, isOutput=False)
    bkp = nc.declare_dram_parameter("bk", [1, C], BF16, isOutput=False)
    bvp = nc.declare_dram_parameter("bv", [1, C], BF16, isOutput=False)
    bop = nc.declare_dram_parameter("bo", [1, C], BF16, isOutput=False)
    b1p = nc.declare_dram_parameter("b1", [1, HID], BF16, isOutput=False)
    b2p = nc.declare_dram_parameter("b2", [1, C], BF16, isOutput=False)
    colw = nc.declare_dram_parameter("colw", [NCH * 128, CHUNK_T * 8], I16, isOutput=False)
    biasP = nc.declare_dram_parameter("biasP", [NCH, 128, CHUNK_T * H], BF16, isOutput=False)
    ohP = nc.declare_dram_parameter("ohP", [NCH, 128, CHUNK_T * 128], BF16, isOutput=False)
    ohTP = nc.declare_dram_parameter("ohTP", [NCH, 128, CHUNK_T * 128], BF16, isOutput=False)
    y_out = nc.declare_dram_parameter("y", [LSH, C], F32, isOutput=True)

    with ExitStack() as ctx:
        tc = ctx.enter_context(tile.TileContext(nc))

        dram = ctx.enter_context(tc.tile_pool(name="dram", bufs=1, space="DRAM"))
        kv_sh = dram.tile([LSH, 2 * C], BF16)
        # chunk-major full table: [NAG][NCORES][LSH4]
        kv_full = dram.tile([NCORES * LSH, 2 * C], BF16)
        poutD = dram.tile([LSH, C], BF16)     # unnormalized att scatter
        x1tD = dram.tile([LSH, C], F32)       # x + att (residual mid)

        # ---------------- constants + weights ----------------
        consts = ctx.enter_context(tc.tile_pool(name="consts", bufs=1))
        ident = consts.tile([128, 128], BF16, tag="ident")
        make_identity(nc, ident[:])
        ones_k1 = consts.tile([1, 128], BF16, tag="ones")
        nc.vector.memset(ones_k1[:], 1.0)
        eps_t = consts.tile([128, 1], F32, tag="eps")
        nc.vector.memset(eps_t[:], EPS)

        wts = ctx.enter_context(tc.tile_pool(name="wts", bufs=1))

        def wload(p, shape, tag):
            t = wts.tile(shape, BF16, tag=tag, name="w_" + tag)
            nc.sync.dma_start(out=t[:], in_=p[:])
            return t

        wq_sb = wload(wq, [128, 4, C], "wq"); wk_sb = wload(wk, [128, 4, C], "wk")
        wv_sb = wload(wv, [128, 4, C], "wv"); wo_sb = wload(wo, [128, 4, C], "wo")
        w1_sb = wload(w1, [128, 4, HID], "w1"); w2_sb = wload(w2, [128, 8, C], "w2")
        bq_sb = wload(bqp, [1, C], "bq"); bk_sb = wload(bkp, [1, C], "bk")
        bv_sb = wload(bvp, [1, C], "bv"); bo_sb = wload(bop, [1, C], "bo")
        b1_sb = wload(b1p, [1, HID], "bb1"); b2_sb = wload(b2p, [1, C], "bb2")

        # q table lives in SBUF for the whole run: [128 rows, NBLK, C] bf16
        qtab_pool = ctx.enter_context(tc.tile_pool(name="qtab", bufs=1))
        q_sb = qtab_pool.tile([128, NBLK, C], BF16)
        # per-block softmax denominators, kept in SBUF until phase C
        psum_pool = ctx.enter_context(tc.tile_pool(name="psums", bufs=1))
        pssum_sb = psum_pool.tile([128, NBLK, H], F32)

        # ---------------- phase B: LN1 (folded), QKV, chunked allgather ----
        with ExitStack() as pctx:
            xap = pctx.enter_context(tc.tile_pool(name="xap", bufs=1))
            xall = xap.tile([128, NBLK, C], F32)
            lnp = pctx.enter_context(tc.tile_pool(name="lnp", bufs=4))
            xnp = pctx.enter_context(tc.tile_pool(name="xnp", bufs=3))
            trp = pctx.enter_context(tc.tile_pool(name="trp", bufs=2, space="PSUM"))
            qkvp = pctx.enter_context(tc.tile_pool(name="qkvp", bufs=2, space="PSUM"))
            obp = pctx.enter_context(tc.tile_pool(name="obp", bufs=3))

            mvall = xap.tile([128, NBLK, 2], F32, tag="mvall")
            for ib in range(NBLK):
                sl = slice(ib * 128, (ib + 1) * 128)
                nc.sync.dma_start(out=xall[:, ib, :], in_=x_c[sl, :])
                stats = lnp.tile([128, 6], F32, tag="l1st")
                nc.vector.bn_stats(stats[:], xall[:, ib, :])
                nc.vector.bn_aggr(mvall[:, ib, :], stats[:])
            sdall = xap.tile([128, NBLK], F32, tag="sdall")
            nc.scalar.activation(sdall[:], mvall[:, :, 1], AF.Sqrt, bias=eps_t[:])
            rsall = xap.tile([128, NBLK], F32, tag="rsall")
            nc.vector.reciprocal(rsall[:], sdall[:])

            for ib in range(NBLK):
                sl = slice(ib * 128, (ib + 1) * 128)
                xnb = xnp.tile([128, C], BF16, tag="xnb")
                nc.vector.tensor_scalar(xnb[:], xall[:, ib, :],
                                        mvall[:, ib, 0:1], rsall[:, ib:ib + 1],
                                        op0=ALU.subtract, op1=ALU.mult)
                xnT = xnp.tile([128, 4, 128], BF16, tag="xnT")
                for g in range(4):
                    pt = trp.tile([128, 128], BF16)
                    nc.tensor.transpose(pt[:], xnb[:, g * 128:(g + 1) * 128], ident[:])
                    nc.scalar.copy(xnT[:, g, :], pt[:])
                for w_sb, bias_sb, dst in (
                    (wq_sb, bq_sb, None),
                    (wk_sb, bk_sb, 0),
                    (wv_sb, bv_sb, 1),
                ):
                    ps = qkvp.tile([128, C], F32)
                    for g in range(4):
                        nc.tensor.matmul(ps[:], lhsT=xnT[:, g, :], rhs=w_sb[:, g, :],
                                         start=(g == 0), stop=False)
                    nc.tensor.matmul(ps[:], lhsT=ones_k1[:], rhs=bias_sb[:],
                                     start=False, stop=True)
                    if dst is None:
                        nc.vector.tensor_copy(q_sb[:, ib, :], ps[:])
                    else:
                        ob = obp.tile([128, C], BF16)
                        nc.scalar.copy(ob[:], ps[:])
                        nc.sync.dma_start(out=kv_sh[sl, dst * C:(dst + 1) * C], in_=ob[:])
                # fire allgather for each finished 1/nag of the shard
                if (ib + 1) % BPA == 0:
                    j = (ib + 1) // BPA - 1
                    nc.gpsimd.collective_compute(
                        "AllGather", ALU.bypass,
                        replica_groups=[list(range(NCORES))],
                        ins=[kv_sh[j * LSH4:(j + 1) * LSH4, :]],
                        outs=[kv_full[j * NCORES * LSH4:(j + 1) * NCORES * LSH4, :]],
                    )

        # ---------------- phase E: edge loop ------------------------------
        # Software pipeline per iteration ch:
        #   dma(ch+1) | acts(ch-1) [Act exp]  | stage1(ch) [PE qg + Act qcp
        #   + DVE prod] | wt(ch-1) [DVE] | tree(ch) [DVE] | scatter(ch-1)
        # The exp for chunk ch runs one iteration later so Act never waits
        # on the DVE reduction mid-chunk.
        with ExitStack() as pctx:
            kvp = pctx.enter_context(tc.tile_pool(name="kvp", bufs=3))
            idxp = pctx.enter_context(tc.tile_pool(name="idxp", bufs=3))
            bp = pctx.enter_context(tc.tile_pool(name="bp", bufs=3))
            ohp_ = pctx.enter_context(tc.tile_pool(name="ohp", bufs=3))
            ohtp = pctx.enter_context(tc.tile_pool(name="ohtp", bufs=3))
            workp = pctx.enter_context(tc.tile_pool(name="workp", bufs=1))
            qcpp = pctx.enter_context(tc.tile_pool(name="qcpp", bufs=2))
            scp = pctx.enter_context(tc.tile_pool(name="scp", bufs=2))
            pexpp = pctx.enter_context(tc.tile_pool(name="pexpp", bufs=2))
            wtp = pctx.enter_context(tc.tile_pool(name="wtp", bufs=2))
            evp = pctx.enter_context(tc.tile_pool(name="evp", bufs=2))
            qgp = pctx.enter_context(tc.tile_pool(name="qgp", bufs=4, space="PSUM"))
            pop_ = pctx.enter_context(tc.tile_pool(name="pout", bufs=2, space="PSUM"))
            psp = pctx.enter_context(tc.tile_pool(name="pssum", bufs=2, space="PSUM"))

            state = {"pout": None, "pssum": None}
            stash = {}

            def _emit_dma(ch):
                tiles_c = min(CHUNK_T, NT - ch * CHUNK_T)
                n_idx = tiles_c * 128
                cidx = idxp.tile([128, CHUNK_T * 8], I16, tag="cidx")
                nc.sync.dma_start(out=cidx[:], in_=colw[ch * 128:(ch + 1) * 128, :])
                kvb = kvp.tile([128, CHUNK_T, 2 * C], BF16)
                nc.gpsimd.dma_gather(
                    out_ap=kvb[:, :tiles_c, :], in_ap=kv_full[0:maxrow[ch], :],
                    idxs_ap=cidx[:, :n_idx // 16],
                    num_idxs=n_idx, num_idxs_reg=n_idx, elem_size=2 * C,
                    single_packet=False)
                bia = bp.tile([128, CHUNK_T, H], BF16, tag="bia")
                nc.sync.dma_start(
                    out=bia[:, :tiles_c, :],
                    in_=biasP[ch, :, :tiles_c * H].rearrange(
                        "p (t h) -> p t h", h=H))
                ohc = ohp_.tile([128, CHUNK_T, 128], BF16, tag="oh")
                nc.sync.dma_start(
                    out=ohc[:, :tiles_c, :],
                    in_=ohP[ch, :, :tiles_c * 128].rearrange(
                        "p (t r) -> p t r", r=128))
                ohtc = ohtp.tile([128, CHUNK_T, 128], BF16, tag="ohT")
                nc.sync.dma_start(
                    out=ohtc[:, :tiles_c, :],
                    in_=ohTP[ch, :, :tiles_c * 128].rearrange(
                        "p (t e) -> p t e", e=128))
                return dict(tiles_c=tiles_c, kvb=kvb, bia=bia, ohc=ohc, ohtc=ohtc)

            def _emit_stage1(ch, dd):
                tc_ = dd["tiles_c"]
                prod = workp.tile([128, CHUNK_T, C], BF16, tag="prod")
                qcp = qcpp.tile([128, CHUNK_T, C], BF16, tag="qcp")
                for slot in range(tc_):
                    t = ch * CHUNK_T + slot
                    rb = t // T_BLK
                    qps = qgp.tile([128, C], F32, tag="qg")
                    nc.tensor.matmul(qps[:], lhsT=dd["ohtc"][:, slot, :],
                                     rhs=q_sb[:, rb, :], start=True, stop=True)
                    nc.scalar.copy(qcp[:, slot, :], qps[:])
                    nc.vector.tensor_tensor(prod[:, slot, :], dd["kvb"][:, slot, 0:C],
                                            qcp[:, slot, :], op=ALU.mult)
                dd["prod"] = prod

            def _emit_tree(ch, dd):
                tc_ = dd["tiles_c"]
                prod = dd["prod"]
                # tree reduce d: 64 -> 32 -> 16 -> 8, then axis-reduce
                pv = prod[:, :tc_, :].rearrange("p t (h d) -> p t h d", h=H)
                lv1 = workp.tile([128, CHUNK_T, H, 32], BF16, tag="lv1")
                nc.vector.tensor_tensor(lv1[:, :tc_, :, :], pv[:, :, :, 0:32],
                                        pv[:, :, :, 32:64], op=ALU.add)
                lv2 = workp.tile([128, CHUNK_T, H, 16], BF16, tag="lv2")
                nc.vector.tensor_tensor(lv2[:, :tc_, :, :], lv1[:, :tc_, :, 0:16],
                                        lv1[:, :tc_, :, 16:32], op=ALU.add)
                lv3 = workp.tile([128, CHUNK_T, H, 8], BF16, tag="lv3")
                nc.vector.tensor_tensor(lv3[:, :tc_, :, :], lv2[:, :tc_, :, 0:8],
                                        lv2[:, :tc_, :, 8:16], op=ALU.add)
                sc = workp.tile([128, CHUNK_T, H], F32, tag="sc")
                nc.vector.tensor_reduce(sc[:, :tc_, :], lv3[:, :tc_, :, :],
                                        axis=AX.X, op=ALU.add)
                sc2 = scp.tile([128, CHUNK_T, H], F32, tag="sc2")
                nc.vector.tensor_tensor(sc2[:, :tc_, :], sc[:, :tc_, :],
                                        dd["bia"][:, :tc_, :], op=ALU.add)
                dd["sc2"] = sc2

            def _emit_acts(ch, dd):
                tc_ = dd["tiles_c"]
                sc2 = dd["sc2"]
                p8c = scp.tile([128, CHUNK_T, H], BF16, tag="p8")
                nc.scalar.activation(p8c[:, :tc_, :], sc2[:, :tc_, :], AF.Exp)
                dd["p8c"] = p8c
                pexp = pexpp.tile([128, CHUNK_T, C], BF16, tag="pexp")
                s2 = sc2[:, :tc_, :]
                src_b = bass.AP(tensor=s2.tensor, offset=s2.offset,
                                ap=[s2.ap[0], s2.ap[1], s2.ap[2], [0, D]])
                nc.scalar.activation(
                    pexp[:, :tc_, :].rearrange("p t (h d) -> p t h d", h=H),
                    src_b, AF.Exp)
                dd["pexp"] = pexp

            def _emit_wt(ch, dd):
                tc_ = dd["tiles_c"]
                wtc = wtp.tile([128, CHUNK_T, C], BF16, tag="wt")
                nc.vector.tensor_tensor(wtc[:, :tc_, 0:C], dd["kvb"][:, :tc_, C:2 * C],
                                        dd["pexp"][:, :tc_, :], op=ALU.mult)
                dd["wtc"] = wtc

            def _emit_scatter(ch, dd):
                for s in range(dd["tiles_c"]):
                    ts_ = ch * CHUNK_T + s
                    rb_, tb_ = divmod(ts_, T_BLK)
                    if tb_ == 0:
                        state["pout"] = pop_.tile([128, C], F32, tag="pout", name="pout")
                        state["pssum"] = psp.tile([128, H], F32, tag="pssum", name="pssum")
                    nc.tensor.matmul(state["pout"][:], lhsT=dd["ohc"][:, s, :],
                                     rhs=dd["wtc"][:, s, :],
                                     start=(tb_ == 0), stop=(tb_ == T_BLK - 1))
                    nc.tensor.matmul(state["pssum"][:], lhsT=dd["ohc"][:, s, :],
                                     rhs=dd["p8c"][:, s, :],
                                     start=(tb_ == 0), stop=(tb_ == T_BLK - 1))
                    if tb_ == T_BLK - 1:
                        # evict: pout -> DRAM (bf16), pssum -> SBUF
                        sl = slice(rb_ * 128, (rb_ + 1) * 128)
                        ev = evp.tile([128, C], BF16, tag="ev")
                        nc.scalar.copy(ev[:], state["pout"][:])
                        nc.sync.dma_start(out=poutD[sl, :], in_=ev[:])
                        nc.vector.tensor_copy(pssum_sb[:, rb_, :],
                                              state["pssum"][:])

            for j in range(min(1, NCH)):
                stash[j] = _emit_dma(j)
            for ch in range(NCH):
                if ch + 1 < NCH:
                    stash[ch + 1] = _emit_dma(ch + 1)
                if ch >= 1:
                    _emit_acts(ch - 1, stash[ch - 1])
                _emit_stage1(ch, stash[ch])
                if ch >= 1:
                    _emit_wt(ch - 1, stash[ch - 1])
                _emit_tree(ch, stash[ch])
                if ch >= 1:
                    _emit_scatter(ch - 1, stash[ch - 1])
                    del stash[ch - 1]
            _emit_acts(NCH - 1, stash[NCH - 1])
            _emit_wt(NCH - 1, stash[NCH - 1])
            _emit_scatter(NCH - 1, stash[NCH - 1])

        # ---------------- phase C: block tails -----------------------------
        # sweep 1: att normalize, Wo, residual, LN2, transpose (Copy+Sqrt
        # act tables); sweep 2: MLP (Silu table) + final residual.
        with ExitStack() as pctx:
            pp = pctx.enter_context(tc.tile_pool(name="pp", bufs=3))
            xbp = pctx.enter_context(tc.tile_pool(name="xbp", bufs=3))
            s1w = pctx.enter_context(tc.tile_pool(name="s1w", bufs=2))
            lnp2 = pctx.enter_context(tc.tile_pool(name="lnp2", bufs=2))
            trp2 = pctx.enter_context(tc.tile_pool(name="trp2", bufs=2, space="PSUM"))
            mmo = pctx.enter_context(tc.tile_pool(name="mmo", bufs=2, space="PSUM"))
            z2Tp = pctx.enter_context(tc.tile_pool(name="z2Tp", bufs=1))
            z2T_all = z2Tp.tile([128, NBLK, 4, 128], BF16)

            for rb in range(NBLK):
                sl = slice(rb * 128, (rb + 1) * 128)
                poutS = pp.tile([128, C], BF16, tag="poutS")
                nc.sync.dma_start(out=poutS[:], in_=poutD[sl, :])
                xb2 = xbp.tile([128, C], F32, tag="xb2")
                nc.sync.dma_start(out=xb2[:], in_=x_c[sl, :])
                sm = lnp2.tile([128, H], F32, tag="sm")
                nc.vector.tensor_scalar(sm[:], pssum_sb[:, rb, :], 1e-30, None,
                                        op0=ALU.max)
                rec = lnp2.tile([128, H], F32, tag="rec")
                nc.vector.reciprocal(rec[:], sm[:])
                rexp = s1w.tile([128, C], BF16, tag="rexp")
                rap = bass.AP(tensor=rec.tensor, offset=rec[:].offset,
                              ap=[rec[:].ap[0], [1, H], [0, D]])
                nc.scalar.activation(
                    rexp[:].rearrange("p (h d) -> p h d", h=H), rap, AF.Copy)
                att = s1w.tile([128, C], BF16, tag="att")
                nc.vector.tensor_tensor(att[:], poutS[:], rexp[:], op=ALU.mult)
                attT = s1w.tile([128, 4, 128], BF16, tag="attT")
                for g in range(4):
                    pt = trp2.tile([128, 128], BF16)
                    nc.tensor.transpose(pt[:], att[:, g * 128:(g + 1) * 128], ident[:])
                    nc.scalar.copy(attT[:, g, :], pt[:])
                po = mmo.tile([128, C], F32, tag="mm")
                for g in range(4):
                    nc.tensor.matmul(po[:], lhsT=attT[:, g, :], rhs=wo_sb[:, g, :],
                                     start=(g == 0), stop=False)
                nc.tensor.matmul(po[:], lhsT=ones_k1[:], rhs=bo_sb[:],
                                 start=False, stop=True)
                x1t = s1w.tile([128, C], F32, tag="x1t")
                nc.vector.tensor_tensor(x1t[:], po[:], xb2[:], op=ALU.add)
                nc.sync.dma_start(out=x1tD[sl, :], in_=x1t[:])
                # LN2 (folded gamma/beta)
                stats = lnp2.tile([128, 6], F32, tag="l2st")
                nc.vector.bn_stats(stats[:], x1t[:])
                mv = lnp2.tile([128, 2], F32, tag="l2mv")
                nc.vector.bn_aggr(mv[:], stats[:])
                sd = lnp2.tile([128, 1], F32, tag="l2sd")
                nc.scalar.activation(sd[:], mv[:, 1:2], AF.Sqrt, bias=eps_t[:])
                rs = lnp2.tile([128, 1], F32, tag="l2rs")
                nc.vector.reciprocal(rs[:], sd[:])
                xn2 = s1w.tile([128, C], BF16, tag="xn2")
                nc.vector.tensor_scalar(xn2[:], x1t[:], mv[:, 0:1], rs[:],
                                        op0=ALU.subtract, op1=ALU.mult)
                for g in range(4):
                    pt = trp2.tile([128, 128], BF16)
                    nc.tensor.transpose(pt[:], xn2[:, g * 128:(g + 1) * 128], ident[:])
                    nc.vector.tensor_copy(z2T_all[:, rb, g, :], pt[:])

            # sweep 2: MLP
            for rb in range(NBLK):
                sl = slice(rb * 128, (rb + 1) * 128)
                x1tS = xbp.tile([128, C], F32, tag="x1tS")
                nc.sync.dma_start(out=x1tS[:], in_=x1tD[sl, :])
                hs = s1w.tile([128, 8, 128], BF16, tag="hs")
                for half in range(2):
                    ph_t = mmo.tile([128, C], F32, tag="mm")
                    ph = ph_t[:].rearrange("p (a b) -> p a b", a=4)
                    for c4 in range(4):
                        chc = half * 4 + c4
                        csl = slice(chc * 128, (chc + 1) * 128)
                        for g in range(4):
                            nc.tensor.matmul(ph[:, c4, :], lhsT=w1_sb[:, g, csl],
                                             rhs=z2T_all[:, rb, g, :],
                                             start=(g == 0), stop=False)
                        nc.tensor.matmul(ph[:, c4, :], lhsT=b1_sb[:, csl],
                                         rhs=ones_k1[:], start=False, stop=True)
                    nc.scalar.activation(hs[:, half * 4:(half + 1) * 4, :],
                                         ph[:, :, :], AF.Silu)
                py = mmo.tile([128, C], F32, tag="mm")
                for chc in range(8):
                    nc.tensor.matmul(py[:], lhsT=hs[:, chc, :], rhs=w2_sb[:, chc, :],
                                     start=(chc == 0), stop=False)
                nc.tensor.matmul(py[:], lhsT=ones_k1[:], rhs=b2_sb[:],
                                 start=False, stop=True)
                yt = s1w.tile([128, C], F32, tag="yt")
                nc.vector.tensor_tensor(yt[:], py[:], x1tS[:], op=ALU.add)
                nc.sync.dma_start(out=y_out[sl, :], in_=yt[:])

    nc.finalize()
    _split_multi_waits(nc)
    return nc


# --------------------------------------------------------------------------
# entry point
# --------------------------------------------------------------------------

def kernel(**inputs) -> np.ndarray:
    x = np.asarray(inputs["x"], np.float32)
    row = np.asarray(inputs["row_index"]).astype(np.int64)
    col = np.asarray(inputs["col_index"]).astype(np.int64)
    att_bias = np.asarray(inputs["att_bias"], np.float32)
    L = x.shape[0]
    LSH = L // NCORES

    T_BLK, NT, NCH, maxrow, cores = _preprocess_edges(L, row, col, att_bias)
    # quantize AG-dep bounds to allgather chunk granularity for caching
    S = max(1, L // max(1, _nag(L // NCORES // 128)))
    maxrow = [min(L, -(-m // S) * S) for m in maxrow]

    w = _prep_weights(inputs)

    key = (L, T_BLK, NT, NCH, tuple(maxrow))
    if key not in _prog_cache:
        _prog_cache[key] = _build_program(L, T_BLK, NT, NCH, maxrow)
    nc = _prog_cache[key]

    in_maps = []
    for c in range(NCORES):
        m = dict(w)
        m["x_c"] = np.ascontiguousarray(x[c * LSH:(c + 1) * LSH])
        m.update(cores[c])
        in_maps.append(m)

    global LAST_EXEC_NS, LAST_RESULTS
    res = run_bass_kernel_spmd(nc, in_maps, list(range(NCORES)), trace=TRACE)
    LAST_RESULTS = res
    LAST_EXEC_NS = res.exec_time_ns
    return np.concatenate([res.results[c]["y"] for c in range(NCORES)], axis=0)


# revision 25
# speedup vs baseline: 1.0538x; 1.0538x over previous
"""Trainium2 Bass kernel for a sparse-attention EncoderLayer.

Sharding: rows (L) split into 8 contiguous shards of L/8; each edge is owned
by the core that owns its destination row (row_index is sorted, so each
core's edges are a contiguous range).  Each core computes Q/K/V for its row
shard; K/V shards are AllGathered (bf16, Shared output, in chunks) so every
core holds the full K/V table in HBM; per-edge K/V rows are fetched with
dma_gather; per-edge Q rows come from a one-hot PE matmul against the
SBUF-resident Q table.  Segment softmax runs without max-subtraction
(scores are bounded, exp cannot overflow in f32).  One-hot row selectors
are precomputed on the host.

v2 layout relative to the first version:
  - LN gamma/beta folded into Wq/Wk/Wv/W1 host-side: the on-chip LN is just
    (x - mu) * rstd.
  - Block tails (att norm, Wo, residual, LN2, MLP) are deferred to a phase C
    after the edge loop: the edge phase evicts the scatter PSUM to DRAM and
    runs a pure Copy/Exp Act stream (no act-table thrash), and phase C runs
    the dense matmuls back-to-back (PE stays at high p-state).
  - CHUNK_T=8 edge tiles per gather chunk; software pipeline with the exp
    stage deferred by one chunk so no engine head-of-line blocks another.
"""

import math
import numpy as np
from contextlib import ExitStack

from ml_dtypes import bfloat16, float8_e4m3

import concourse.bass as bass
import concourse.mybir as mybir
import concourse.tile as tile
from concourse import bacc
from concourse.bass_utils import run_bass_kernel_spmd
from concourse.masks import make_identity

NCORES = 8
C, H, D, HID = 512, 8, 64, 1024
EPS = 1e-5
CHUNK_T = 8   # edge tiles (of 128 edges) per dma_gather chunk
NAG = 8       # allgather chunks
F32 = mybir.dt.float32
BF16 = mybir.dt.bfloat16
FP8 = mybir.dt.float8e4
I16 = mybir.dt.int16
DR = mybir.MatmulPerfMode.DoubleRow
AF = mybir.ActivationFunctionType
ALU = mybir.AluOpType
AX = mybir.AxisListType

_prog_cache = {}
TRACE = False
LAST_EXEC_NS = None
LAST_RESULTS = None


# --------------------------------------------------------------------------
# host-side preprocessing
# --------------------------------------------------------------------------

def _nag(NBLK):
    return NAG if NBLK % NAG == 0 else 1


def _wrap_idx(idx):
    """[n] int -> [128, n//16] int16, wrapped (idx i at partition i%16,
    column i//16) and replicated across the 8 Q7 cores."""
    n = idx.shape[0]
    w = np.ascontiguousarray(idx.reshape(n // 16, 16).T).astype(np.int16)
    return np.tile(w, (8, 1))


def _preprocess_edges(L, row, col, att_bias):
    LSH = L // NCORES
    NBLK = LSH // 128
    bounds = np.searchsorted(row, np.arange(NCORES + 1) * LSH)

    per_core = []
    t_blk = 1
    for c in range(NCORES):
        e0, e1 = int(bounds[c]), int(bounds[c + 1])
        r = row[e0:e1] - c * LSH
        blk = r >> 7
        cnt = np.bincount(blk, minlength=NBLK)
        t_blk = max(t_blk, int(np.max((cnt + 127) // 128)) if len(cnt) else 1)
        per_core.append((e0, e1, r, blk, cnt))

    T_BLK = t_blk
    NT = NBLK * T_BLK
    NCH = (NT + CHUNK_T - 1) // CHUNK_T
    NTP = NCH * CHUNK_T
    LSH4 = LSH // _nag(NBLK)

    cores = []
    for c in range(NCORES):
        e0, e1, r, blk, cnt = per_core[c]
        ne = e1 - e0
        starts = np.zeros(NBLK, dtype=np.int64)
        np.cumsum(cnt[:-1], out=starts[1:])

        npad = NTP * 128
        # col: global node id -> kv_full row (allgather chunk-major layout)
        gcol = col[e0:e1]
        oc, loc = gcol // LSH, gcol % LSH
        kvrow = (loc // LSH4) * (NCORES * LSH4) + oc * LSH4 + (loc % LSH4)
        # order edges within each block by kv row: improves gather locality
        # and lets early chunks depend on only a prefix of the allgather
        perm = np.lexsort((kvrow, blk))
        blk_s = blk[perm]
        kvrow_s = kvrow[perm]
        idx_in_blk = np.arange(ne, dtype=np.int64) - starts[blk_s]
        dst = blk_s * (T_BLK * 128) + idx_in_blk

        colP = np.zeros(npad, dtype=np.int64)
        rlocP = np.zeros(npad, dtype=np.int64)
        biasP = np.full((npad, H), -30000.0, dtype=np.float32)
        colP[dst] = kvrow_s
        rlocP[dst] = r[perm] & 127
        biasP[dst] = att_bias[e0:e1][perm]
        # per-chunk upper bound on referenced kv rows (for partial AG deps)
        maxrow = colP.reshape(NCH, CHUNK_T * 128).max(axis=1) + 1

        colw = _wrap_idx(colP).reshape(128, NCH, CHUNK_T * 8).transpose(1, 0, 2)
        colw = colw.reshape(NCH * 128, CHUNK_T * 8)
        # one-hot row selector per edge, chunk-partition-major for contiguous
        # DMA: oh[t, e, r] (scatter lhsT, fp8); only real edges are set.
        ohu = np.zeros((NTP * 128, 128), dtype=np.uint8)
        ohu[dst, rlocP[dst]] = 0x38  # fp8e4m3 1.0
        oh = (ohu.view(float8_e4m3).reshape(NCH, CHUNK_T, 128, 128)
              .transpose(0, 2, 1, 3).reshape(NCH, 128, CHUNK_T * 128))
        # ohT[t, r, e]: row-partition (q-gather lhsT, fp8); set for ALL padded
        # slots too (col 0 row 0) so no garbage — padded p is 0 via bias.
        e_in_t = np.arange(npad, dtype=np.int64) % 128
        ohTu = np.zeros((NTP * 128, 128), dtype=np.uint8)
        ohTu[(np.arange(npad) // 128) * 128 + rlocP, e_in_t] = 0x38
        ohT = (ohTu.view(float8_e4m3).reshape(NCH, CHUNK_T, 128, 128)
               .transpose(0, 2, 1, 3).reshape(NCH, 128, CHUNK_T * 128))
        # bias, chunk-partition-major bf16: [NCH, 128, CHUNK_T*H]
        biasT = (biasP.reshape(NCH, CHUNK_T, 128, H).transpose(0, 2, 1, 3)
                 .reshape(NCH, 128, CHUNK_T * H).astype(bfloat16))
        cores.append(dict(
            colw=np.ascontiguousarray(colw),
            biasP=np.ascontiguousarray(biasT),
            ohP=np.ascontiguousarray(oh),
            ohTP=np.ascontiguousarray(ohT),
        ))
        cores[-1]["_maxrow"] = maxrow
    # chunk AG-dep bound must be identical across cores (same program):
    maxrow_all = np.max([c.pop("_maxrow") for c in cores], axis=0)
    return T_BLK, NT, NCH, [int(x) for x in maxrow_all], cores


def _prep_weights(inp):
    scale = 1.0 / math.sqrt(D)
    g1 = np.asarray(inp["ln1_g"], np.float32)
    b1 = np.asarray(inp["ln1_b"], np.float32)
    g2 = np.asarray(inp["ln2_g"], np.float32)
    b2 = np.asarray(inp["ln2_b"], np.float32)

    def mat(w, kchunks, dt=bfloat16):
        w = np.asarray(w, np.float32)
        k, n = w.shape
        assert k == kchunks * 128
        return np.ascontiguousarray(
            w.reshape(kchunks, 128, n).transpose(1, 0, 2)).astype(dt)

    def rowv(b):
        return np.asarray(b, np.float32)[None, :].astype(bfloat16)

    Wq = np.asarray(inp["Wq"], np.float32)
    Wk = np.asarray(inp["Wk"], np.float32)
    Wv = np.asarray(inp["Wv"], np.float32)
    W1 = np.asarray(inp["W1"], np.float32)
    W2 = np.asarray(inp["W2"], np.float32)

    # LN gamma/beta folded into the projections (z = xn*g + b):
    #   z @ W + bw  ==  xn @ (g[:,None]*W)  +  (b @ W + bw)
    # fp8 weights are stored x16 (into e4m3's normal range); the x1/16
    # descale rides the PSUM->SBUF copy / the silu input scale.
    WS = 16.0
    return dict(
        wq=mat(g1[:, None] * Wq * scale * WS, 4, float8_e4m3),
        wk=mat(g1[:, None] * Wk * WS, 4, float8_e4m3),
        wv=mat(g1[:, None] * Wv * WS, 4, float8_e4m3),
        wo=mat(inp["Wo"], 4),
        w1=mat(g2[:, None] * W1 * WS, 4, float8_e4m3),
        w2=mat(W2 * WS, 8, float8_e4m3),
        bq=rowv((b1 @ Wq + np.asarray(inp["bq"], np.float32)) * scale * WS),
        bk=rowv((b1 @ Wk + np.asarray(inp["bk"], np.float32)) * WS),
        bv=rowv((b1 @ Wv + np.asarray(inp["bv"], np.float32)) * WS),
        bo=rowv(inp["bo"]),
        b1=rowv((b2 @ W1 + np.asarray(inp["b1"], np.float32)) * WS),
        b2=rowv(np.asarray(inp["b2"], np.float32) * WS),
    )


# --------------------------------------------------------------------------
# walrus workaround: split Drain instructions carrying >1 sem wait
# --------------------------------------------------------------------------

def _split_multi_waits(nc):
    nid = [0]
    for fn in nc.m.functions:
        for blk in fn.blocks:
            insts = blk.instructions
            i = 0
            while i < len(insts):
                inst = insts[i]
                si = inst.sync_info
                if (isinstance(inst, mybir.InstDrain)
                        and si is not None and si.on_wait and len(si.on_wait) > 1):
                    waits = list(si.on_wait)
                    nops = []
                    for w in waits[:-1]:
                        nid[0] += 1
                        nops.append(mybir.InstNoOp(
                            name=f"I-waitfix-{nid[0]}",
                            engine=inst.engine, ins=[], outs=[],
                            sync_info=mybir.SyncInfo(on_wait=[w], on_update=[]),
                        ))
                    inst.sync_info = mybir.SyncInfo(
                        on_wait=[waits[-1]], on_update=list(si.on_update))
                    insts[i:i] = nops
                    i += len(nops)
                i += 1


# --------------------------------------------------------------------------
# device program
# --------------------------------------------------------------------------

def _build_program(L, T_BLK, NT, NCH, maxrow):
    LSH = L // NCORES
    NBLK = LSH // 128
    nag = _nag(NBLK)
    LSH4 = LSH // nag
    BPA = NBLK // nag  # blocks per allgather chunk
    nc = bacc.Bacc(num_devices=NCORES)

    x_c = nc.declare_dram_parameter("x_c", [LSH, C], F32, isOutput=False)
    wq = nc.declare_dram_parameter("wq", [128, 4, C], FP8, isOutput=False)
    wk = nc.declare_dram_parameter("wk", [128, 4, C], FP8, isOutput=False)
    wv = nc.declare_dram_parameter("wv", [128, 4, C], FP8, isOutput=False)
    wo = nc.declare_dram_parameter("wo", [128, 4, C], BF16, isOutput=False)
    w1 = nc.declare_dram_parameter("w1", [128, 4, HID], FP8, isOutput=False)
    w2 = nc.declare_dram_parameter("w2", [128, 8, C], FP8, isOutput=False)
    bqp = nc.declare_dram_parameter("bq", [1, C], BF16, isOutput=False)
    bkp = nc.declare_dram_parameter("bk", [1, C], BF16, isOutput=False)
    bvp = nc.declare_dram_parameter("bv", [1, C], BF16, isOutput=False)
    bop = nc.declare_dram_parameter("bo", [1, C], BF16, isOutput=False)
    b1p = nc.declare_dram_parameter("b1", [1, HID], BF16, isOutput=False)
    b2p = nc.declare_dram_parameter("b2", [1, C], BF16, isOutput=False)
    colw = nc.declare_dram_parameter("colw", [NCH * 128, CHUNK_T * 8], I16, isOutput=False)
    biasP = nc.declare_dram_parameter("biasP", [NCH, 128, CHUNK_T * H], BF16, isOutput=False)
    ohP = nc.declare_dram_parameter("ohP", [NCH, 128, CHUNK_T * 128], FP8, isOutput=False)
    ohTP = nc.declare_dram_parameter("ohTP", [NCH, 128, CHUNK_T * 128], FP8, isOutput=False)
    y_out = nc.declare_dram_parameter("y", [LSH, C], F32, isOutput=True)

    with ExitStack() as ctx:
        tc = ctx.enter_context(tile.TileContext(nc))

        dram = ctx.enter_context(tc.tile_pool(name="dram", bufs=1, space="DRAM"))
        kv_sh = dram.tile([LSH, 2 * C], BF16)
        # chunk-major full table: [NAG][NCORES][LSH4]
        kv_full = dram.tile([NCORES * LSH, 2 * C], BF16)
        poutD = dram.tile([LSH, C], BF16)     # unnormalized att scatter
        x1tD = dram.tile([LSH, C], F32)       # x + att (residual mid)

        # ---------------- constants + weights ----------------
        consts = ctx.enter_context(tc.tile_pool(name="consts", bufs=1))
        ident = consts.tile([128, 128], BF16, tag="ident")
        make_identity(nc, ident[:])
        ident8 = consts.tile([128, 128], FP8, tag="ident8")
        make_identity(nc, ident8[:])
        ones_k1 = consts.tile([1, 128], BF16, tag="ones")
        nc.vector.memset(ones_k1[:], 1.0)
        eps_t = consts.tile([128, 1], F32, tag="eps")
        nc.vector.memset(eps_t[:], EPS)

        wts = ctx.enter_context(tc.tile_pool(name="wts", bufs=1))

        def wload(p, shape, tag, dt=BF16):
            t = wts.tile(shape, dt, tag=tag, name="w_" + tag)
            nc.sync.dma_start(out=t[:], in_=p[:])
            return t

        wq_sb = wload(wq, [128, 4, C], "wq", FP8)
        wk_sb = wload(wk, [128, 4, C], "wk", FP8)
        wv_sb = wload(wv, [128, 4, C], "wv", FP8)
        wo_sb = wload(wo, [128, 4, C], "wo")
        w1_sb = wload(w1, [128, 4, HID], "w1", FP8)
        w2_sb = wload(w2, [128, 8, C], "w2", FP8)
        bq_sb = wload(bqp, [1, C], "bq"); bk_sb = wload(bkp, [1, C], "bk")
        bv_sb = wload(bvp, [1, C], "bv"); bo_sb = wload(bop, [1, C], "bo")
        b1_sb = wload(b1p, [1, HID], "bb1"); b2_sb = wload(b2p, [1, C], "bb2")

        # q table lives in SBUF for the whole run: [128 rows, NBLK, C] fp8
        # (values are exact selections via the fp8 one-hot gather matmul)
        qtab_pool = ctx.enter_context(tc.tile_pool(name="qtab", bufs=1))
        q_sb = qtab_pool.tile([128, NBLK, C], FP8)
        # per-block softmax denominators, kept in SBUF until phase C
        psum_pool = ctx.enter_context(tc.tile_pool(name="psums", bufs=1))
        pssum_sb = psum_pool.tile([128, NBLK, H], F32)

        # ---------------- phase B: LN1 (folded), QKV, chunked allgather ----
        with ExitStack() as pctx:
            xap = pctx.enter_context(tc.tile_pool(name="xap", bufs=1))
            xall = xap.tile([128, NBLK, C], F32)
            lnp = pctx.enter_context(tc.tile_pool(name="lnp", bufs=4))
            xnp = pctx.enter_context(tc.tile_pool(name="xnp", bufs=3))
            trp = pctx.enter_context(tc.tile_pool(name="trp", bufs=2, space="PSUM"))
            qkvp = pctx.enter_context(tc.tile_pool(name="qkvp", bufs=2, space="PSUM"))
            obp = pctx.enter_context(tc.tile_pool(name="obp", bufs=3))

            mvall = xap.tile([128, NBLK, 2], F32, tag="mvall")
            for ib in range(NBLK):
                sl = slice(ib * 128, (ib + 1) * 128)
                nc.sync.dma_start(out=xall[:, ib, :], in_=x_c[sl, :])
                stats = lnp.tile([128, 6], F32, tag="l1st")
                nc.vector.bn_stats(stats[:], xall[:, ib, :])
                nc.vector.bn_aggr(mvall[:, ib, :], stats[:])
            sdall = xap.tile([128, NBLK], F32, tag="sdall")
            nc.scalar.activation(sdall[:], mvall[:, :, 1], AF.Sqrt, bias=eps_t[:])
            rsall = xap.tile([128, NBLK], F32, tag="rsall")
            nc.vector.reciprocal(rsall[:], sdall[:])

            for ib in range(NBLK):
                sl = slice(ib * 128, (ib + 1) * 128)
                xnb = xnp.tile([128, C], BF16, tag="xnb")
                nc.vector.tensor_scalar(xnb[:], xall[:, ib, :],
                                        mvall[:, ib, 0:1], rsall[:, ib:ib + 1],
                                        op0=ALU.subtract, op1=ALU.mult)
                xnT = xnp.tile([128, 4, 128], FP8, tag="xnT")
                for g in range(4):
                    pt = trp.tile([128, 128], BF16)
                    nc.tensor.transpose(pt[:], xnb[:, g * 128:(g + 1) * 128], ident[:])
                    nc.scalar.copy(xnT[:, g, :], pt[:])
                for w_sb, bias_sb, dst in (
                    (wq_sb, bq_sb, None),
                    (wk_sb, bk_sb, 0),
                    (wv_sb, bv_sb, 1),
                ):
                    ps = qkvp.tile([128, C], F32)
                    for j in range(2):
                        nc.tensor.matmul(ps[:], lhsT=xnT[:, 2 * j:2 * j + 2, :],
                                         rhs=w_sb[:, 2 * j:2 * j + 2, :],
                                         perf_mode=DR, start=(j == 0), stop=False)
                    nc.tensor.matmul(ps[:], lhsT=ones_k1[:], rhs=bias_sb[:],
                                     start=False, stop=True)
                    if dst is None:
                        # q_sb holds q_true*4 (fp8 normal range); the /4
                        # rides the edge-phase PSUM->bf16 copy.
                        nc.vector.tensor_scalar(q_sb[:, ib, :], ps[:],
                                                4.0 / 16.0, None, op0=ALU.mult)
                    else:
                        ob = obp.tile([128, C], BF16)
                        nc.scalar.mul(ob[:], ps[:], 1.0 / 16.0)
                        nc.sync.dma_start(out=kv_sh[sl, dst * C:(dst + 1) * C], in_=ob[:])
                # fire allgather for each finished 1/nag of the shard
                if (ib + 1) % BPA == 0:
                    j = (ib + 1) // BPA - 1
                    nc.gpsimd.collective_compute(
                        "AllGather", ALU.bypass,
                        replica_groups=[list(range(NCORES))],
                        ins=[kv_sh[j * LSH4:(j + 1) * LSH4, :]],
                        outs=[kv_full[j * NCORES * LSH4:(j + 1) * NCORES * LSH4, :]],
                    )

        # ---------------- phase E: edge loop ------------------------------
        # Software pipeline per iteration ch:
        #   dma(ch+1) | acts(ch-1) [Act exp]  | stage1(ch) [PE qg + Act qcp
        #   + DVE prod] | wt(ch-1) [DVE] | tree(ch) [DVE] | scatter(ch-1)
        # The exp for chunk ch runs one iteration later so Act never waits
        # on the DVE reduction mid-chunk.
        with ExitStack() as pctx:
            kvp = pctx.enter_context(tc.tile_pool(name="kvp", bufs=3))
            idxp = pctx.enter_context(tc.tile_pool(name="idxp", bufs=3))
            bp = pctx.enter_context(tc.tile_pool(name="bp", bufs=3))
            ohp_ = pctx.enter_context(tc.tile_pool(name="ohp", bufs=3))
            ohtp = pctx.enter_context(tc.tile_pool(name="ohtp", bufs=3))
            workp = pctx.enter_context(tc.tile_pool(name="workp", bufs=1))
            qcpp = pctx.enter_context(tc.tile_pool(name="qcpp", bufs=2))
            scp = pctx.enter_context(tc.tile_pool(name="scp", bufs=2))
            pexpp = pctx.enter_context(tc.tile_pool(name="pexpp", bufs=2))
            wtp = pctx.enter_context(tc.tile_pool(name="wtp", bufs=2))
            evp = pctx.enter_context(tc.tile_pool(name="evp", bufs=2))
            qgp = pctx.enter_context(tc.tile_pool(name="qgp", bufs=2, space="PSUM"))
            pop_ = pctx.enter_context(tc.tile_pool(name="pout", bufs=2, space="PSUM"))
            psp = pctx.enter_context(tc.tile_pool(name="pssum", bufs=2, space="PSUM"))

            state = {"pout": None, "pssum": None}
            stash = {}

            def _emit_dma(ch):
                tiles_c = min(CHUNK_T, NT - ch * CHUNK_T)
                n_idx = tiles_c * 128
                cidx = idxp.tile([128, CHUNK_T * 8], I16, tag="cidx")
                nc.sync.dma_start(out=cidx[:], in_=colw[ch * 128:(ch + 1) * 128, :])
                kvb = kvp.tile([128, CHUNK_T, 2 * C], BF16)
                nc.gpsimd.dma_gather(
                    out_ap=kvb[:, :tiles_c, :], in_ap=kv_full[0:maxrow[ch], :],
                    idxs_ap=cidx[:, :n_idx // 16],
                    num_idxs=n_idx, num_idxs_reg=n_idx, elem_size=2 * C,
                    single_packet=False)
                bia = bp.tile([128, CHUNK_T, H], BF16, tag="bia")
                nc.sync.dma_start(
                    out=bia[:, :tiles_c, :],
                    in_=biasP[ch, :, :tiles_c * H].rearrange(
                        "p (t h) -> p t h", h=H))
                ohc = ohp_.tile([128, CHUNK_T, 128], FP8, tag="oh")
                nc.sync.dma_start(
                    out=ohc[:, :tiles_c, :],
                    in_=ohP[ch, :, :tiles_c * 128].rearrange(
                        "p (t r) -> p t r", r=128))
                ohtc = ohtp.tile([128, CHUNK_T, 128], FP8, tag="ohT")
                nc.sync.dma_start(
                    out=ohtc[:, :tiles_c, :],
                    in_=ohTP[ch, :, :tiles_c * 128].rearrange(
                        "p (t e) -> p t e", e=128))
                return dict(tiles_c=tiles_c, kvb=kvb, bia=bia, ohc=ohc, ohtc=ohtc)

            def _emit_stage1(ch, dd):
                tc_ = dd["tiles_c"]
                prod = workp.tile([128, CHUNK_T, C], BF16, tag="prod")
                qcp = qcpp.tile([128, CHUNK_T, C], BF16, tag="qcp")
                for pair in range((tc_ + 1) // 2):
                    n = min(2, tc_ - 2 * pair)
                    qps = qgp.tile([128, 2, C], F32, tag="qg")
                    for j in range(n):
                        slot = 2 * pair + j
                        t = ch * CHUNK_T + slot
                        rb = t // T_BLK
                        nc.tensor.matmul(qps[:, j, :], lhsT=dd["ohtc"][:, slot, :],
                                         rhs=q_sb[:, rb, :], start=True, stop=True)
                    # q_sb holds q_true*4: descale on the PSUM->bf16 copy
                    nc.scalar.mul(qcp[:, 2 * pair:2 * pair + n, :],
                                  qps[:, 0:n, :], 0.25)
                nc.vector.tensor_tensor(prod[:, :tc_, :], dd["kvb"][:, :tc_, 0:C],
                                        qcp[:, :tc_, :], op=ALU.mult)
                dd["prod"] = prod

            def _emit_tree(ch, dd):
                tc_ = dd["tiles_c"]
                prod = dd["prod"]
                # tree reduce d: 64 -> 32 -> 16 -> 8, then axis-reduce
                pv = prod[:, :tc_, :].rearrange("p t (h d) -> p t h d", h=H)
                lv1 = workp.tile([128, CHUNK_T, H, 32], BF16, tag="lv1")
                nc.vector.tensor_tensor(lv1[:, :tc_, :, :], pv[:, :, :, 0:32],
                                        pv[:, :, :, 32:64], op=ALU.add)
                lv2 = workp.tile([128, CHUNK_T, H, 16], BF16, tag="lv2")
                nc.vector.tensor_tensor(lv2[:, :tc_, :, :], lv1[:, :tc_, :, 0:16],
                                        lv1[:, :tc_, :, 16:32], op=ALU.add)
                lv3 = workp.tile([128, CHUNK_T, H, 8], BF16, tag="lv3")
                nc.vector.tensor_tensor(lv3[:, :tc_, :, :], lv2[:, :tc_, :, 0:8],
                                        lv2[:, :tc_, :, 8:16], op=ALU.add)
                sc = workp.tile([128, CHUNK_T, H], F32, tag="sc")
                nc.vector.tensor_reduce(sc[:, :tc_, :], lv3[:, :tc_, :, :],
                                        axis=AX.X, op=ALU.add)
                sc2 = scp.tile([128, CHUNK_T, H], F32, tag="sc2")
                nc.vector.tensor_tensor(sc2[:, :tc_, :], sc[:, :tc_, :],
                                        dd["bia"][:, :tc_, :], op=ALU.add)
                dd["sc2"] = sc2

            def _emit_acts(ch, dd):
                # exp broadcast at width 16 only (the wt multiply re-reads it
                # 4x): Act cost drops 3.5x vs a full-width broadcast.
                tc_ = dd["tiles_c"]
                sc2 = dd["sc2"]
                W16 = 16
                pexp = pexpp.tile([128, CHUNK_T, H, W16], BF16, tag="pexp")
                s2 = sc2[:, :tc_, :]
                src_b = bass.AP(tensor=s2.tensor, offset=s2.offset,
                                ap=[s2.ap[0], s2.ap[1], s2.ap[2], [0, W16]])
                nc.scalar.activation(pexp[:, :tc_, :, :], src_b, AF.Exp)
                dd["pexp"] = pexp

            def _emit_wt(ch, dd):
                tc_ = dd["tiles_c"]
                W16 = 16
                wtc = wtp.tile([128, CHUNK_T, C], BF16, tag="wt")
                wv_ = wtc[:, :tc_, :].rearrange("p t (h d) -> p t h d", h=H)
                vv = dd["kvb"][:, :tc_, C:2 * C].rearrange("p t (h d) -> p t h d", h=H)
                pe16 = dd["pexp"][:, :tc_, :, :]
                for j in range(D // W16):
                    nc.vector.tensor_tensor(wv_[:, :, :, j * W16:(j + 1) * W16],
                                            vv[:, :, :, j * W16:(j + 1) * W16],
                                            pe16, op=ALU.mult)
                dd["wtc"] = wtc

            def _emit_scatter(ch, dd):
                for s in range(dd["tiles_c"]):
                    ts_ = ch * CHUNK_T + s
                    rb_, tb_ = divmod(ts_, T_BLK)
                    if tb_ == 0:
                        state["pout"] = pop_.tile([128, C], F32, tag="pout", name="pout")
                        state["pssum"] = psp.tile([128, H], F32, tag="pssum", name="pssum")
                    nc.tensor.matmul(state["pout"][:], lhsT=dd["ohc"][:, s, :],
                                     rhs=dd["wtc"][:, s, :],
                                     start=(tb_ == 0), stop=(tb_ == T_BLK - 1))
                    nc.tensor.matmul(state["pssum"][:], lhsT=dd["ohc"][:, s, :],
                                     rhs=dd["pexp"][:, s, :, 0:1],
                                     start=(tb_ == 0), stop=(tb_ == T_BLK - 1))
                    if tb_ == T_BLK - 1:
                        # evict: pout -> DRAM (bf16), pssum -> SBUF
                        sl = slice(rb_ * 128, (rb_ + 1) * 128)
                        ev = evp.tile([128, C], BF16, tag="ev")
                        nc.scalar.copy(ev[:], state["pout"][:])
                        nc.sync.dma_start(out=poutD[sl, :], in_=ev[:])
                        nc.vector.tensor_copy(pssum_sb[:, rb_, :],
                                              state["pssum"][:])

            for j in range(min(1, NCH)):
                stash[j] = _emit_dma(j)
            for ch in range(NCH):
                if ch + 1 < NCH:
                    stash[ch + 1] = _emit_dma(ch + 1)
                if ch >= 1:
                    _emit_acts(ch - 1, stash[ch - 1])
                _emit_stage1(ch, stash[ch])
                if ch >= 1:
                    _emit_wt(ch - 1, stash[ch - 1])
                _emit_tree(ch, stash[ch])
                if ch >= 1:
                    _emit_scatter(ch - 1, stash[ch - 1])
                    del stash[ch - 1]
            _emit_acts(NCH - 1, stash[NCH - 1])
            _emit_wt(NCH - 1, stash[NCH - 1])
            _emit_scatter(NCH - 1, stash[NCH - 1])

        # ---------------- phase C: block tails -----------------------------
        # sweep 1: att normalize, Wo, residual, LN2, transpose (Copy+Sqrt
        # act tables); sweep 2: MLP (Silu table) + final residual.
        with ExitStack() as pctx:
            pp = pctx.enter_context(tc.tile_pool(name="pp", bufs=3))
            xbp = pctx.enter_context(tc.tile_pool(name="xbp", bufs=3))
            s1w = pctx.enter_context(tc.tile_pool(name="s1w", bufs=2))
            lnp2 = pctx.enter_context(tc.tile_pool(name="lnp2", bufs=2))
            trp2 = pctx.enter_context(tc.tile_pool(name="trp2", bufs=2, space="PSUM"))
            mmo = pctx.enter_context(tc.tile_pool(name="mmo", bufs=2, space="PSUM"))
            z2Tp = pctx.enter_context(tc.tile_pool(name="z2Tp", bufs=1))
            z2T_all = z2Tp.tile([128, NBLK, 4, 128], FP8)

            for rb in range(NBLK):
                sl = slice(rb * 128, (rb + 1) * 128)
                poutS = pp.tile([128, C], BF16, tag="poutS")
                nc.sync.dma_start(out=poutS[:], in_=poutD[sl, :])
                xb2 = xbp.tile([128, C], F32, tag="xb2")
                nc.sync.dma_start(out=xb2[:], in_=x_c[sl, :])
                sm = lnp2.tile([128, H], F32, tag="sm")
                nc.vector.tensor_scalar(sm[:], pssum_sb[:, rb, :], 1e-30, None,
                                        op0=ALU.max)
                rec = lnp2.tile([128, H], F32, tag="rec")
                nc.vector.reciprocal(rec[:], sm[:])
                rexp = s1w.tile([128, C], BF16, tag="rexp")
                rap = bass.AP(tensor=rec.tensor, offset=rec[:].offset,
                              ap=[rec[:].ap[0], [1, H], [0, D]])
                nc.scalar.activation(
                    rexp[:].rearrange("p (h d) -> p h d", h=H), rap, AF.Copy)
                att = s1w.tile([128, C], BF16, tag="att")
                nc.vector.tensor_tensor(att[:], poutS[:], rexp[:], op=ALU.mult)
                attT = s1w.tile([128, 4, 128], BF16, tag="attT")
                for g in range(4):
                    pt = trp2.tile([128, 128], BF16)
                    nc.tensor.transpose(pt[:], att[:, g * 128:(g + 1) * 128], ident[:])
                    nc.scalar.copy(attT[:, g, :], pt[:])
                po = mmo.tile([128, C], F32, tag="mm")
                for g in range(4):
                    nc.tensor.matmul(po[:], lhsT=attT[:, g, :], rhs=wo_sb[:, g, :],
                                     start=(g == 0), stop=False)
                nc.tensor.matmul(po[:], lhsT=ones_k1[:], rhs=bo_sb[:],
                                 start=False, stop=True)
                x1t = s1w.tile([128, C], F32, tag="x1t")
                nc.vector.tensor_tensor(x1t[:], po[:], xb2[:], op=ALU.add)
                nc.sync.dma_start(out=x1tD[sl, :], in_=x1t[:])
                # LN2 (folded gamma/beta)
                stats = lnp2.tile([128, 6], F32, tag="l2st")
                nc.vector.bn_stats(stats[:], x1t[:])
                mv = lnp2.tile([128, 2], F32, tag="l2mv")
                nc.vector.bn_aggr(mv[:], stats[:])
                sd = lnp2.tile([128, 1], F32, tag="l2sd")
                nc.scalar.activation(sd[:], mv[:, 1:2], AF.Sqrt, bias=eps_t[:])
                rs = lnp2.tile([128, 1], F32, tag="l2rs")
                nc.vector.reciprocal(rs[:], sd[:])
                xn2 = s1w.tile([128, C], BF16, tag="xn2")
                nc.vector.tensor_scalar(xn2[:], x1t[:], mv[:, 0:1], rs[:],
                                        op0=ALU.subtract, op1=ALU.mult)
                for g in range(4):
                    pt = trp2.tile([128, 128], BF16)
                    nc.tensor.transpose(pt[:], xn2[:, g * 128:(g + 1) * 128], ident[:])
                    nc.vector.tensor_copy(z2T_all[:, rb, g, :], pt[:])

            # sweep 2: MLP
            for rb in range(NBLK):
                sl = slice(rb * 128, (rb + 1) * 128)
                x1tS = xbp.tile([128, C], F32, tag="x1tS")
                nc.sync.dma_start(out=x1tS[:], in_=x1tD[sl, :])
                hs = s1w.tile([128, 8, 128], FP8, tag="hs")
                for half in range(2):
                    ph_t = mmo.tile([128, C], F32, tag="mm")
                    ph = ph_t[:].rearrange("p (a b) -> p a b", a=4)
                    for c4 in range(4):
                        chc = half * 4 + c4
                        csl = slice(chc * 128, (chc + 1) * 128)
                        for j in range(2):
                            nc.tensor.matmul(ph[:, c4, :],
                                             lhsT=w1_sb[:, 2 * j:2 * j + 2, csl],
                                             rhs=z2T_all[:, rb, 2 * j:2 * j + 2, :],
                                             perf_mode=DR, start=(j == 0), stop=False)
                        nc.tensor.matmul(ph[:, c4, :], lhsT=b1_sb[:, csl],
                                         rhs=ones_k1[:], start=False, stop=True)
                    # W1 is stored x16: descale on the silu input
                    nc.scalar.activation(hs[:, half * 4:(half + 1) * 4, :],
                                         ph[:, :, :], AF.Silu, scale=1.0 / 16.0)
                py = mmo.tile([128, C], F32, tag="mm")
                for j in range(4):
                    nc.tensor.matmul(py[:], lhsT=hs[:, 2 * j:2 * j + 2, :],
                                     rhs=w2_sb[:, 2 * j:2 * j + 2, :],
                                     perf_mode=DR, start=(j == 0), stop=False)
                nc.tensor.matmul(py[:], lhsT=ones_k1[:], rhs=b2_sb[:],
                                 start=False, stop=True)
                yt = s1w.tile([128, C], F32, tag="yt")
                nc.vector.scalar_tensor_tensor(yt[:], py[:], 1.0 / 16.0, x1tS[:],
                                               op0=ALU.mult, op1=ALU.add)
                nc.sync.dma_start(out=y_out[sl, :], in_=yt[:])

    nc.finalize()
    _split_multi_waits(nc)
    return nc


# --------------------------------------------------------------------------
# entry point
# --------------------------------------------------------------------------

def kernel(**inputs) -> np.ndarray:
    x = np.asarray(inputs["x"], np.float32)
    row = np.asarray(inputs["row_index"]).astype(np.int64)
    col = np.asarray(inputs["col_index"]).astype(np.int64)
    att_bias = np.asarray(inputs["att_bias"], np.float32)
    L = x.shape[0]
    LSH = L // NCORES

    T_BLK, NT, NCH, maxrow, cores = _preprocess_edges(L, row, col, att_bias)
    # quantize AG-dep bounds to allgather chunk granularity for caching
    S = max(1, L // max(1, _nag(L // NCORES // 128)))
    maxrow = [min(L, -(-m // S) * S) for m in maxrow]

    w = _prep_weights(inputs)

    key = (L, T_BLK, NT, NCH, tuple(maxrow))
    if key not in _prog_cache:
        _prog_cache[key] = _build_program(L, T_BLK, NT, NCH, maxrow)
    nc = _prog_cache[key]

    in_maps = []
    for c in range(NCORES):
        m = dict(w)
        m["x_c"] = np.ascontiguousarray(x[c * LSH:(c + 1) * LSH])
        m.update(cores[c])
        in_maps.append(m)

    global LAST_EXEC_NS, LAST_RESULTS
    res = run_bass_kernel_spmd(nc, in_maps, list(range(NCORES)), trace=TRACE)
    LAST_RESULTS = res
    LAST_EXEC_NS = res.exec_time_ns
    return np.concatenate([res.results[c]["y"] for c in range(NCORES)], axis=0)
